# revision 9
# baseline (speedup 1.0000x reference)
"""Trainium2 Bass kernel for a 2-layer bidirectional LSTM encoder applied as a
single cell step from zero state, vectorized over (B, T).

Math (per reference): e = emb[x]; for each (layer, dir):
    g = inp @ W_ih.T + b_ih + b_hh   (gate order i,f,g,o; f unused since c0=0)
    c = sigmoid(i) * tanh(g) ; h = sigmoid(o) * tanh(c)
W_hh never contributes (h0 = 0), so it is not even loaded.

Sharding: data-parallel over batch. B=32 -> 4 batches (1024 rows) per core
across 8 cores; weights/embedding replicated. No collectives. Per-core program:
  phase 1: indirect-DMA gather of the 1024 embedding rows -> e [rows, E]
  phase 2: PE-transpose e -> eT [E, rows] (cast to fp32r)
  phase 3: layer 0 both dirs, gates in [gate, row] layout (lhsT = W_igo.T
           tiles, moving = eT); per-partition bias applied for free by ACT
           sigmoid/tanh; h written transposed -> inp1T = [h0f; h0b].T
  phase 4: enc0 = h0f+h0b, PE-transpose back to row-major, DMA out
  phase 5: layer 1 both dirs with lhsT = inp1T slices (stationary), moving
           operand = W1_igo.T tiles -> gates in [row, gate] layout; bias via a
           partition-broadcast [128, 1536] tile added on DVE; outputs
           row-major; enc1/h_last/c_last extracted along the way.

Matmuls run in float32r (fast fp32 path on the PE, ~1e-4 rel err).
"""
import os
import sys
import types

sys.path.insert(0, "/opt/trn_rl_repo")

import numpy as np

# Provide antenv.axon_hooks (NTFF profile hook registry) if the image's antenv
# stub lacks it — needed for trace=True timing under axon.
try:
    import antenv.axon_hooks  # noqa: F401
except ImportError:
    import antenv

    _m = types.ModuleType("antenv.axon_hooks")
    _m._hook = None

    def _set_hook(hook):
        _m._hook = hook

    def _get_hook():
        if _m._hook is None:
            try:
                from trn_agent_boot.trn_boot import _ntff_profile_via_ctypes

                _m._hook = _ntff_profile_via_ctypes("/opt/axon/libaxon_pjrt.so")
            except Exception:
                pass
        return _m._hook

    _m.set_axon_ntff_profile_hook = _set_hook
    _m.get_axon_ntff_profile_hook = _get_hook
    sys.modules["antenv.axon_hooks"] = _m
    antenv.axon_hooks = _m

import concourse.bass as bass
import concourse.bacc as bacc
import concourse.mybir as mybir
import concourse.tile as tile
from concourse.bass_utils import run_bass_kernel_spmd
import concourse.bass_utils as _bass_utils

if not getattr(_bass_utils, "_ldw_opt_patched", False):
    _orig_run_command = _bass_utils.run_command

    def _run_command_ldw(argv, **kwargs):
        argv = ["--enable-ldw-opt=true" if a == "--enable-ldw-opt=false" else a
                for a in argv]
        return _orig_run_command(argv, **kwargs)

    _bass_utils.run_command = _run_command_ldw
    _bass_utils._ldw_opt_patched = True
from concourse.masks import make_identity
from contextlib import ExitStack

P = 128
B, T, E, H, V = 32, 256, 512, 512, 50000
NCORES = 8
NB = B // NCORES          # batches per core
R = NB * T                # rows per core (1024)
NM = R // P               # row tiles per core (8)
G = 3 * H                 # i,g,o gates kept (1536)
KE = E // P               # layer-0 k-tiles (4)
K1 = 2 * H // P           # layer-1 k-tiles (8)
NCH = R // 512            # 512-wide row chunks (2)
F32 = mybir.dt.float32
F32R = mybir.dt.float32r
SIG = mybir.ActivationFunctionType.Sigmoid
TANH = mybir.ActivationFunctionType.Tanh

_PROGRAM = None  # cached Bacc program — build once per process
LAST_RESULTS = None  # BassKernelResults of the most recent run (for test.py)


def _build_program():
    nc = bacc.Bacc("TRN2", target_bir_lowering=False, debug=False)

    x_d = nc.dram_tensor("x", [NM, P], mybir.dt.int32, kind="ExternalInput").ap()
    emb_d = nc.dram_tensor("emb", [V, E], F32, kind="ExternalInput").ap()
    wt0_d = nc.dram_tensor("wt0", [2, E, G], F32R, kind="ExternalInput").ap()
    b0i_d = nc.dram_tensor("b0i", [2, G], F32, kind="ExternalInput").ap()
    b0h_d = nc.dram_tensor("b0h", [2, G], F32, kind="ExternalInput").ap()
    wt1_d = nc.dram_tensor("wt1", [2, 2 * H, G], F32R, kind="ExternalInput").ap()
    b1i_d = nc.dram_tensor("b1i", [2, G], F32, kind="ExternalInput").ap()
    b1h_d = nc.dram_tensor("b1h", [2, G], F32, kind="ExternalInput").ap()

    enc0_d = nc.dram_tensor("enc0", [R, H], F32, kind="ExternalOutput").ap()
    enc1_d = nc.dram_tensor("enc1", [R, H], F32, kind="ExternalOutput").ap()
    # [p, k*NB+b] = value at h-dim k*128+p, local batch b
    h0l_d = nc.dram_tensor("h0l", [P, KE * NB], F32, kind="ExternalOutput").ap()
    c0l_d = nc.dram_tensor("c0l", [P, KE * NB], F32, kind="ExternalOutput").ap()
    h1l_d = nc.dram_tensor("h1l", [NB, H], F32, kind="ExternalOutput").ap()
    c1l_d = nc.dram_tensor("c1l", [NB, H], F32, kind="ExternalOutput").ap()

    with tile.TileContext(nc) as tc, ExitStack() as ctx:
        const = ctx.enter_context(tc.tile_pool(name="const", bufs=1))
        persist = ctx.enter_context(tc.tile_pool(name="persist", bufs=1))
        misc = ctx.enter_context(tc.tile_pool(name="misc", bufs=1))
        act = ctx.enter_context(tc.tile_pool(name="act", bufs=2))
        outp = ctx.enter_context(tc.tile_pool(name="outp", bufs=2))
        psg = ctx.enter_context(tc.tile_pool(name="psg", bufs=6, space="PSUM"))
        pst = ctx.enter_context(tc.tile_pool(name="pst", bufs=2, space="PSUM"))

        # persistent activations:
        # inp1T: [feature % 128 (part), kk*R + row] for k-tile kk; kk 0-3 =
        # h0f.T, kk 4-7 = h0b.T. fp32r: it is the layer-1 stationary operand.
        inp1T = persist.tile([P, K1 * R], F32R)
        h1f_all = persist.tile([P, NM * H], F32)
        enc0T = persist.tile([P, KE * R], F32)

        # h_last/c_last staging for layer 0: [p, k*NB+b]
        h0lt = const.tile([P, KE * NB], F32, tag="h0lt")
        c0lt = const.tile([P, KE * NB], F32, tag="c0lt")

        w1pre = ctx.enter_context(tc.tile_pool(name="w1pre", bufs=1))

        with ExitStack() as scope_a:
            epool = scope_a.enter_context(tc.tile_pool(name="epool", bufs=1))
            etpool = scope_a.enter_context(tc.tile_pool(name="etpool", bufs=1))
            w0pool = scope_a.enter_context(tc.tile_pool(name="w0pool", bufs=1))

            # phase 1 first in program order: the serialized gpsimd gathers are
            # the critical path to the first matmul. idx loads, then gathers,
            # then weight streams; bias/identity setup overlaps the gathers.
            e_ts = []
            for m in range(NM):
                idx_t = misc.tile([P, 1], mybir.dt.int32, tag=f"idx{m}")
                nc.sync.dma_start(out=idx_t[:], in_=x_d[m].unsqueeze(1))
                e_t = epool.tile([P, E], F32, tag=f"e{m % 4}")
                nc.gpsimd.indirect_dma_start(
                    out=e_t[:], out_offset=None, in_=emb_d[:],
                    in_offset=bass.IndirectOffsetOnAxis(ap=idx_t[:, :1], axis=0),
                )
                e_ts.append(e_t)

            # layer-0 weights: all 8 (d, k) tiles resident so the d=1 pass
            # streams in during d=0 compute with no PE stall
            w0 = [[None] * KE for _ in range(2)]
            for d in range(2):
                for k in range(KE):
                    wt = w0pool.tile([P, G], F32R, tag=f"w0_{d}_{k}")
                    nc.sync.dma_start(out=wt[:], in_=wt0_d[d, k * P:(k + 1) * P, :])
                    w0[d][k] = wt

            # prefetch layer-1 dir-0 k0/k1 weight tiles during layer 0
            w1pre_t = []
            for k in range(2):
                wt = w1pre.tile([P, G], F32R, tag=f"w1pre_{k}", name=f"w1pre_{k}")
                nc.sync.dma_start(out=wt[:], in_=wt1_d[0, k * P:(k + 1) * P, :])
                w1pre_t.append(wt)

            ident = const.tile([P, P], F32)
            make_identity(nc, ident)

            # layer-0 per-partition biases [128, 12]: col m=gate*4+hm
            b0sum = []
            for d in range(2):
                t_i = misc.tile([P, G // P], F32, tag="b0i")
                nc.sync.dma_start(out=t_i[:], in_=b0i_d[d].rearrange("(m p) -> p m", p=P))
                s = const.tile([P, G // P], F32, tag=f"b0sum{d}")
                nc.sync.dma_start(out=s[:], in_=b0h_d[d].rearrange("(m p) -> p m", p=P))
                nc.vector.tensor_add(s[:], s[:], t_i[:])
                b0sum.append(s)

            # layer-1 broadcast biases [128, 1536]: b_ih then += b_hh via a
            # SWDGE accumulate DMA (no temp tile, no DVE pass)
            b128 = []
            for d in range(2):
                bb = const.tile([P, G], F32, tag=f"b128_{d}")
                nc.sync.dma_start(out=bb[:],
                                  in_=b1i_d[d].unsqueeze(0).partition_broadcast(P))
                nc.gpsimd.dma_start(out=bb[:],
                                    in_=b1h_d[d].unsqueeze(0).partition_broadcast(P),
                                    accum_op=mybir.AluOpType.add)
                b128.append(bb)

            # phase 2: PE-transpose into eT [E (4 ptiles), R]. Each 512-wide eT
            # chunk is written by ONE copy (single producer for matmul rhs).
            eT = etpool.tile([P, KE * R], F32R)
            for n in range(NCH):
                for k in range(KE):
                    tp = pst.tile([P, 512], F32, tag="tp")
                    for mm in range(4):
                        nc.tensor.transpose(out=tp[:, mm * P:(mm + 1) * P],
                                            in_=e_ts[n * 4 + mm][:, k * P:(k + 1) * P],
                                            identity=ident[:])
                    nc.vector.tensor_copy(
                        out=eT[:, k * R + n * 512: k * R + (n + 1) * 512], in_=tp[:])

            # phase 3: layer 0, gates in [gate, row] layout
            for d in range(2):
                for hm in range(KE):
                    # interleave both row chunks so consecutive matmuls share
                    # the stationary operand (walrus ldw-opt elides reloads);
                    # the very first group goes chunk-by-chunk instead so the
                    # PE starts before the second gather wave lands
                    pss = [[None] * 3 for _ in range(NCH)]
                    if d == 0 and hm == 0:
                        for n in range(NCH):
                            for gate in range(3):
                                m = gate * 4 + hm
                                pss[n][gate] = psg.tile([P, 512], F32, tag="ps", name=f"ps_{n}_{gate}")
                                for k in range(KE):
                                    nc.tensor.matmul(
                                        out=pss[n][gate][:],
                                        lhsT=w0[d][k][:, m * P:(m + 1) * P],
                                        rhs=eT[:, k * R + n * 512: k * R + (n + 1) * 512],
                                        start=(k == 0), stop=(k == KE - 1),
                                    )
                    else:
                        for gate in range(3):
                            m = gate * 4 + hm
                            for k in range(KE):
                                for n in range(NCH):
                                    if k == 0:
                                        pss[n][gate] = psg.tile([P, 512], F32, tag="ps", name=f"ps_{n}_{gate}")
                                    nc.tensor.matmul(
                                        out=pss[n][gate][:],
                                        lhsT=w0[d][k][:, m * P:(m + 1) * P],
                                        rhs=eT[:, k * R + n * 512: k * R + (n + 1) * 512],
                                        start=(k == 0), stop=(k == KE - 1),
                                    )
                    for n in range(NCH):
                        ps3 = pss[n]
                        si = act.tile([P, 512], F32, tag="si")
                        nc.scalar.activation(si[:], ps3[0][:], SIG,
                                             bias=b0sum[d][:, hm:hm + 1])
                        tg = act.tile([P, 512], F32, tag="tg")
                        nc.scalar.activation(tg[:], ps3[1][:], TANH,
                                             bias=b0sum[d][:, 4 + hm:5 + hm])
                        c_t = act.tile([P, 512], F32, tag="c")
                        nc.vector.tensor_mul(c_t[:], si[:], tg[:])
                        if d == 0:
                            # c_last lives at rows b*256+255 -> batches 2n,2n+1
                            # at in-chunk columns 255 and 511
                            for j in range(2):
                                b_ = 2 * n + j
                                col = j * 256 + 255
                                nc.vector.tensor_copy(
                                    out=c0lt[:, hm * NB + b_: hm * NB + b_ + 1],
                                    in_=c_t[:, col:col + 1])
                        tc_t = act.tile([P, 512], F32, tag="tc")
                        nc.scalar.activation(tc_t[:], c_t[:], TANH)
                        so = act.tile([P, 512], F32, tag="so")
                        nc.scalar.activation(so[:], ps3[2][:], SIG,
                                             bias=b0sum[d][:, 8 + hm:9 + hm])
                        kk = d * 4 + hm
                        h_ap = inp1T[:, kk * R + n * 512: kk * R + (n + 1) * 512]
                        nc.vector.tensor_mul(h_ap, so[:], tc_t[:])
                        if d == 1:
                            # enc0 chunk ready as soon as both dirs done: gives
                            # the PE transpose work during the L0->L1 handoff
                            base = hm * R + n * 512
                            nc.vector.tensor_add(
                                enc0T[:, base:base + 512],
                                inp1T[:, base:base + 512].bitcast(F32),
                                inp1T[:, KE * R + base: KE * R + base + 512].bitcast(F32))

        # phase 4: enc0 -> row-major -> DMA
        for m in range(NM):
            rm = outp.tile([P, H], F32, tag="rm")
            tp = pst.tile([P, 512], F32, tag="tp")
            for k in range(KE):
                nc.tensor.transpose(out=tp[:, k * P:(k + 1) * P],
                                    in_=enc0T[:, k * R + m * P: k * R + (m + 1) * P],
                                    identity=ident[:])
            nc.vector.tensor_copy(out=rm[:], in_=tp[:])
            nc.sync.dma_start(out=enc0_d[m * P:(m + 1) * P, :], in_=rm[:])

        # phase 5: layer-0 h_last
        for k in range(KE):
            for b_ in range(NB):
                col = k * R + b_ * T + (T - 1)
                nc.vector.tensor_copy(out=h0lt[:, k * NB + b_: k * NB + b_ + 1],
                                      in_=inp1T[:, col:col + 1].bitcast(F32))
        nc.sync.dma_start(out=h0l_d[:], in_=h0lt[:])
        nc.sync.dma_start(out=c0l_d[:], in_=c0lt[:])

        # phase 6: layer 1
        with ExitStack() as scope_c:
            w1pool = scope_c.enter_context(tc.tile_pool(name="w1pool", bufs=1))
            for d in range(2):
                w1 = []
                for k in range(K1):
                    if d == 0 and k < 2:
                        w1.append(w1pre_t[k])
                        continue
                    wt = w1pool.tile([P, G], F32R, tag=f"w1_{k}", name=f"w1_{d}_{k}")
                    nc.sync.dma_start(out=wt[:], in_=wt1_d[d, k * P:(k + 1) * P, :])
                    w1.append(wt)
                for m in range(NM):
                    ps3 = [None] * 3
                    for k in range(K1):
                        for gate in range(3):
                            if k == 0:
                                ps3[gate] = psg.tile([P, 512], F32, tag="ps", name=f"ps1_{gate}")
                            nc.tensor.matmul(
                                out=ps3[gate][:],
                                lhsT=inp1T[:, k * R + m * P: k * R + (m + 1) * P],
                                rhs=w1[k][:, gate * 512:(gate + 1) * 512],
                                start=(k == 0), stop=(k == K1 - 1),
                            )
                    gs = []  # i, g, o post-bias gate tiles [128 rows, 512]
                    for gate, tag in ((0, "si"), (1, "tg"), (2, "so")):
                        g_t = act.tile([P, 512], F32, tag=tag)
                        nc.vector.tensor_add(g_t[:], ps3[gate][:],
                                             b128[d][:, gate * 512:(gate + 1) * 512])
                        gs.append(g_t)
                    # sigmoid/tanh applied in place on the gate tiles
                    nc.scalar.activation(gs[0][:], gs[0][:], SIG)
                    nc.scalar.activation(gs[1][:], gs[1][:], TANH)
                    c_t = act.tile([P, 512], F32, tag="c")
                    nc.vector.tensor_mul(c_t[:], gs[0][:], gs[1][:])
                    tc_t = act.tile([P, 512], F32, tag="tc")
                    nc.scalar.activation(tc_t[:], c_t[:], TANH)
                    nc.scalar.activation(gs[2][:], gs[2][:], SIG)
                    if d == 0:
                        h_ap = h1f_all[:, m * H:(m + 1) * H]
                        nc.vector.tensor_mul(h_ap, gs[2][:], tc_t[:])
                        if m % 2 == 1:  # row m*128+127 = batch (m-1)//2 last step
                            b_ = (m - 1) // 2
                            nc.sync.dma_start(out=h1l_d[b_:b_ + 1, :],
                                              in_=h1f_all[127:128, m * H:(m + 1) * H])
                            nc.sync.dma_start(out=c1l_d[b_:b_ + 1, :],
                                              in_=c_t[127:128, :])
                    else:
                        h1b = act.tile([P, 512], F32, tag="c")
                        nc.vector.tensor_mul(h1b[:], gs[2][:], tc_t[:])
                        e1 = outp.tile([P, H], F32, tag="e1")
                        nc.vector.tensor_add(e1[:], h1f_all[:, m * H:(m + 1) * H],
                                             h1b[:])
                        nc.sync.dma_start(out=enc1_d[m * P:(m + 1) * P, :], in_=e1[:])

    nc.compile()
    return nc


def _prep_inputs(x, emb, W_ih_l0, b_ih_l0, b_hh_l0, W_ih_l1, b_ih_l1, b_hh_l1):
    """Host-side layout prep (slicing / transposition only) + per-core shards."""
    igo = np.r_[0:H, 2 * H:3 * H, 3 * H:4 * H]
    wt0 = np.ascontiguousarray(W_ih_l0[:, igo, :].transpose(0, 2, 1))  # (2, E, G)
    wt1 = np.ascontiguousarray(W_ih_l1[:, igo, :].transpose(0, 2, 1))  # (2, 2H, G)
    b0i = np.ascontiguousarray(b_ih_l0[:, igo])
    b0h = np.ascontiguousarray(b_hh_l0[:, igo])
    b1i = np.ascontiguousarray(b_ih_l1[:, igo])
    b1h = np.ascontiguousarray(b_hh_l1[:, igo])
    in_maps = []
    for i in range(NCORES):
        x_i = np.ascontiguousarray(
            x[i * NB:(i + 1) * NB].reshape(NM, P).astype(np.int32))
        in_maps.append({
            "x": x_i, "emb": emb,
            "wt0": wt0, "b0i": b0i, "b0h": b0h,
            "wt1": wt1, "b1i": b1i, "b1h": b1h,
        })
    return in_maps


def kernel(x, emb, W_ih_l0, W_hh_l0, b_ih_l0, b_hh_l0,
           W_ih_l1, W_hh_l1, b_ih_l1, b_hh_l1):
    global _PROGRAM, LAST_RESULTS
    if _PROGRAM is None:
        _PROGRAM = _build_program()
    nc = _PROGRAM
    in_maps = _prep_inputs(np.asarray(x),
                           np.ascontiguousarray(emb, dtype=np.float32),
                           np.asarray(W_ih_l0, dtype=np.float32),
                           np.asarray(b_ih_l0, dtype=np.float32),
                           np.asarray(b_hh_l0, dtype=np.float32),
                           np.asarray(W_ih_l1, dtype=np.float32),
                           np.asarray(b_ih_l1, dtype=np.float32),
                           np.asarray(b_hh_l1, dtype=np.float32))
    trace = bool(int(os.environ.get("KERNEL_TRACE", "0")))
    res = run_bass_kernel_spmd(nc, in_maps, core_ids=list(range(NCORES)),
                               trace=trace)
    LAST_RESULTS = res

    enc = np.empty((B, T, 2, H), dtype=np.float32)
    h_last = np.empty((2, B, H), dtype=np.float32)
    c_last = np.empty((2, B, H), dtype=np.float32)
    for i in range(NCORES):
        r = res.results[i]
        bs = slice(i * NB, (i + 1) * NB)
        enc[bs, :, 0, :] = r["enc0"].reshape(NB, T, H)
        enc[bs, :, 1, :] = r["enc1"].reshape(NB, T, H)
        # h0l/c0l: [p, k*NB+b] -> [b, k*128+p]
        h_last[0, bs] = r["h0l"].reshape(P, KE, NB).transpose(2, 1, 0).reshape(NB, H)
        c_last[0, bs] = r["c0l"].reshape(P, KE, NB).transpose(2, 1, 0).reshape(NB, H)
        h_last[1, bs] = r["h1l"]
        c_last[1, bs] = r["c1l"]
    return h_last, c_last, enc


# revision 10
# speedup vs baseline: 1.0622x; 1.0622x over previous
"""Trainium2 Bass kernel for a 2-layer bidirectional LSTM encoder applied as a
single cell step from zero state, vectorized over (B, T).

Math (per reference): e = emb[x]; for each (layer, dir):
    g = inp @ W_ih.T + b_ih + b_hh   (gate order i,f,g,o; f unused since c0=0)
    c = sigmoid(i) * tanh(g) ; h = sigmoid(o) * tanh(c)
W_hh never contributes (h0 = 0), so it is not even loaded.

Sharding: data-parallel over batch. B=32 -> 4 batches (1024 rows) per core
across 8 cores; weights/embedding replicated. No collectives. Per-core program:
  phase 1: indirect-DMA gather of the 1024 embedding rows -> e [rows, E]
  phase 2: PE-transpose e -> eT [E, rows] (cast to fp32r)
  phase 3: layer 0 both dirs, gates in [gate, row] layout (lhsT = W_igo.T
           tiles, moving = eT); per-partition bias applied for free by ACT
           sigmoid/tanh; h written transposed -> inp1T = [h0f; h0b].T
  phase 4: enc0 = h0f+h0b, PE-transpose back to row-major, DMA out
  phase 5: layer 1 both dirs with lhsT = inp1T slices (stationary), moving
           operand = W1_igo.T tiles -> gates in [row, gate] layout; bias via a
           partition-broadcast [128, 1536] tile added on DVE; outputs
           row-major; enc1/h_last/c_last extracted along the way.

Matmuls run in float32r (fast fp32 path on the PE, ~1e-4 rel err).
"""
import os
import sys
import types

sys.path.insert(0, "/opt/trn_rl_repo")

import numpy as np

# Provide antenv.axon_hooks (NTFF profile hook registry) if the image's antenv
# stub lacks it — needed for trace=True timing under axon.
try:
    import antenv.axon_hooks  # noqa: F401
except ImportError:
    import antenv

    _m = types.ModuleType("antenv.axon_hooks")
    _m._hook = None

    def _set_hook(hook):
        _m._hook = hook

    def _get_hook():
        if _m._hook is None:
            try:
                from trn_agent_boot.trn_boot import _ntff_profile_via_ctypes

                _m._hook = _ntff_profile_via_ctypes("/opt/axon/libaxon_pjrt.so")
            except Exception:
                pass
        return _m._hook

    _m.set_axon_ntff_profile_hook = _set_hook
    _m.get_axon_ntff_profile_hook = _get_hook
    sys.modules["antenv.axon_hooks"] = _m
    antenv.axon_hooks = _m

import concourse.bass as bass
import concourse.bacc as bacc
import concourse.mybir as mybir
import concourse.tile as tile
from concourse.bass_utils import run_bass_kernel_spmd
import concourse.bass_utils as _bass_utils

if not getattr(_bass_utils, "_ldw_opt_patched", False):
    _orig_run_command = _bass_utils.run_command

    def _run_command_ldw(argv, **kwargs):
        argv = ["--enable-ldw-opt=true" if a == "--enable-ldw-opt=false" else a
                for a in argv]
        return _orig_run_command(argv, **kwargs)

    _bass_utils.run_command = _run_command_ldw
    _bass_utils._ldw_opt_patched = True
from concourse.masks import make_identity
from contextlib import ExitStack

P = 128
B, T, E, H, V = 32, 256, 512, 512, 50000
NCORES = 8
NB = B // NCORES          # batches per core
R = NB * T                # rows per core (1024)
NM = R // P               # row tiles per core (8)
G = 3 * H                 # i,g,o gates kept (1536)
KE = E // P               # layer-0 k-tiles (4)
K1 = 2 * H // P           # layer-1 k-tiles (8)
NCH = R // 512            # 512-wide row chunks (2)
F32 = mybir.dt.float32
F32R = mybir.dt.float32r
SIG = mybir.ActivationFunctionType.Sigmoid
TANH = mybir.ActivationFunctionType.Tanh

_PROGRAM = None  # cached Bacc program — build once per process
LAST_RESULTS = None  # BassKernelResults of the most recent run (for test.py)


def _build_program():
    nc = bacc.Bacc("TRN2", target_bir_lowering=False, debug=False)

    x_d = nc.dram_tensor("x", [NM, P], mybir.dt.int32, kind="ExternalInput").ap()
    emb_d = nc.dram_tensor("emb", [V, E], F32, kind="ExternalInput").ap()
    wt0_d = nc.dram_tensor("wt0", [2, E, G], F32R, kind="ExternalInput").ap()
    b0i_d = nc.dram_tensor("b0i", [2, G], F32, kind="ExternalInput").ap()
    b0h_d = nc.dram_tensor("b0h", [2, G], F32, kind="ExternalInput").ap()
    wt1_d = nc.dram_tensor("wt1", [2, 2 * H, G], F32R, kind="ExternalInput").ap()
    b1i_d = nc.dram_tensor("b1i", [2, G], F32, kind="ExternalInput").ap()
    b1h_d = nc.dram_tensor("b1h", [2, G], F32, kind="ExternalInput").ap()

    enc0_d = nc.dram_tensor("enc0", [R, H], F32, kind="ExternalOutput").ap()
    enc1_d = nc.dram_tensor("enc1", [R, H], F32, kind="ExternalOutput").ap()
    # [p, k*NB+b] = value at h-dim k*128+p, local batch b
    h0l_d = nc.dram_tensor("h0l", [P, KE * NB], F32, kind="ExternalOutput").ap()
    c0l_d = nc.dram_tensor("c0l", [P, KE * NB], F32, kind="ExternalOutput").ap()
    h1l_d = nc.dram_tensor("h1l", [NB, H], F32, kind="ExternalOutput").ap()
    c1l_d = nc.dram_tensor("c1l", [NB, H], F32, kind="ExternalOutput").ap()

    with tile.TileContext(nc) as tc, ExitStack() as ctx:
        const = ctx.enter_context(tc.tile_pool(name="const", bufs=1))
        persist = ctx.enter_context(tc.tile_pool(name="persist", bufs=1))
        misc = ctx.enter_context(tc.tile_pool(name="misc", bufs=1))
        act = ctx.enter_context(tc.tile_pool(name="act", bufs=2))
        outp = ctx.enter_context(tc.tile_pool(name="outp", bufs=2))
        psg = ctx.enter_context(tc.tile_pool(name="psg", bufs=6, space="PSUM"))
        pst = ctx.enter_context(tc.tile_pool(name="pst", bufs=2, space="PSUM"))

        # persistent activations:
        # inp1T: [feature % 128 (part), kk*R + row] for k-tile kk; kk 0-3 =
        # h0f.T, kk 4-7 = h0b.T. fp32r: it is the layer-1 stationary operand.
        inp1T = persist.tile([P, K1 * R], F32R)
        h1f_all = persist.tile([P, NM * H], F32)
        enc0T = persist.tile([P, KE * R], F32)

        # h_last/c_last staging for layer 0: [p, k*NB+b]
        h0lt = const.tile([P, KE * NB], F32, tag="h0lt")
        c0lt = const.tile([P, KE * NB], F32, tag="c0lt")

        w1pre = ctx.enter_context(tc.tile_pool(name="w1pre", bufs=1))

        with ExitStack() as scope_a:
            epool = scope_a.enter_context(tc.tile_pool(name="epool", bufs=1))
            etpool = scope_a.enter_context(tc.tile_pool(name="etpool", bufs=1))
            w0pool = scope_a.enter_context(tc.tile_pool(name="w0pool", bufs=1))

            # phase 1 first in program order: the serialized gpsimd gathers are
            # the critical path to the first matmul. idx loads, then gathers,
            # then weight streams; bias/identity setup overlaps the gathers.
            e_ts = []
            for m in range(NM):
                idx_t = misc.tile([P, 1], mybir.dt.int32, tag=f"idx{m}")
                nc.sync.dma_start(out=idx_t[:], in_=x_d[m].unsqueeze(1))
                e_t = epool.tile([P, E], F32, tag=f"e{m % 4}")
                nc.gpsimd.indirect_dma_start(
                    out=e_t[:], out_offset=None, in_=emb_d[:],
                    in_offset=bass.IndirectOffsetOnAxis(ap=idx_t[:, :1], axis=0),
                )
                e_ts.append(e_t)

            # layer-0 weights: all 8 (d, k) tiles resident so the d=1 pass
            # streams in during d=0 compute with no PE stall
            w0 = [[None] * KE for _ in range(2)]
            for d in range(2):
                for k in range(KE):
                    wt = w0pool.tile([P, G], F32R, tag=f"w0_{d}_{k}")
                    nc.sync.dma_start(out=wt[:], in_=wt0_d[d, k * P:(k + 1) * P, :])
                    w0[d][k] = wt

            ident = const.tile([P, P], F32)
            make_identity(nc, ident)

            # layer-0 per-partition biases [128, 12]: col m=gate*4+hm
            b0sum = []
            for d in range(2):
                t_i = misc.tile([P, G // P], F32, tag="b0i")
                nc.sync.dma_start(out=t_i[:], in_=b0i_d[d].rearrange("(m p) -> p m", p=P))
                s = const.tile([P, G // P], F32, tag=f"b0sum{d}")
                nc.sync.dma_start(out=s[:], in_=b0h_d[d].rearrange("(m p) -> p m", p=P))
                nc.vector.tensor_add(s[:], s[:], t_i[:])
                b0sum.append(s)

            # layer-1 broadcast biases [128, 1536]
            b128 = []
            for d in range(2):
                bb = const.tile([P, G], F32, tag=f"b128_{d}")
                nc.sync.dma_start(out=bb[:],
                                  in_=b1i_d[d].unsqueeze(0).partition_broadcast(P))
                bt = misc.tile([P, G], F32, tag="b1tmp")
                nc.sync.dma_start(out=bt[:],
                                  in_=b1h_d[d].unsqueeze(0).partition_broadcast(P))
                nc.vector.tensor_add(bb[:], bb[:], bt[:])
                b128.append(bb)

            # phase 2: PE-transpose into eT [E (4 ptiles), R]. Each 512-wide eT
            # chunk is written by ONE copy (single producer for matmul rhs).
            eT = etpool.tile([P, KE * R], F32R)
            for n in range(NCH):
                for k in range(KE):
                    tp = pst.tile([P, 512], F32, tag="tp")
                    for mm in range(4):
                        nc.tensor.transpose(out=tp[:, mm * P:(mm + 1) * P],
                                            in_=e_ts[n * 4 + mm][:, k * P:(k + 1) * P],
                                            identity=ident[:])
                    nc.vector.tensor_copy(
                        out=eT[:, k * R + n * 512: k * R + (n + 1) * 512], in_=tp[:])

            # phase 3: layer 0, gates in [gate, row] layout
            w1pre_t = []
            for d in range(2):
                if d == 1:
                    # prefetch layer-1 dir-0 k0/k1 weight tiles now: DMA queue
                    # is quiet and their SBUF lives outside the scope-A stack
                    for k in range(2):
                        wt = w1pre.tile([P, G], F32R, tag=f"w1pre_{k}", name=f"w1pre_{k}")
                        nc.sync.dma_start(out=wt[:], in_=wt1_d[0, k * P:(k + 1) * P, :])
                        w1pre_t.append(wt)
                for hm in range(KE):
                    # interleave both row chunks so consecutive matmuls share
                    # the stationary operand (walrus ldw-opt elides reloads);
                    # the very first group goes chunk-by-chunk instead so the
                    # PE starts before the second gather wave lands
                    pss = [[None] * 3 for _ in range(NCH)]
                    if d == 0 and hm == 0:
                        for n in range(NCH):
                            for gate in range(3):
                                m = gate * 4 + hm
                                pss[n][gate] = psg.tile([P, 512], F32, tag="ps", name=f"ps_{n}_{gate}")
                                for k in range(KE):
                                    nc.tensor.matmul(
                                        out=pss[n][gate][:],
                                        lhsT=w0[d][k][:, m * P:(m + 1) * P],
                                        rhs=eT[:, k * R + n * 512: k * R + (n + 1) * 512],
                                        start=(k == 0), stop=(k == KE - 1),
                                    )
                    else:
                        for gate in range(3):
                            m = gate * 4 + hm
                            for k in range(KE):
                                for n in range(NCH):
                                    if k == 0:
                                        pss[n][gate] = psg.tile([P, 512], F32, tag="ps", name=f"ps_{n}_{gate}")
                                    nc.tensor.matmul(
                                        out=pss[n][gate][:],
                                        lhsT=w0[d][k][:, m * P:(m + 1) * P],
                                        rhs=eT[:, k * R + n * 512: k * R + (n + 1) * 512],
                                        start=(k == 0), stop=(k == KE - 1),
                                    )
                    for n in range(NCH):
                        ps3 = pss[n]
                        si = act.tile([P, 512], F32, tag="si")
                        nc.scalar.activation(si[:], ps3[0][:], SIG,
                                             bias=b0sum[d][:, hm:hm + 1])
                        tg = act.tile([P, 512], F32, tag="tg")
                        nc.scalar.activation(tg[:], ps3[1][:], TANH,
                                             bias=b0sum[d][:, 4 + hm:5 + hm])
                        c_t = act.tile([P, 512], F32, tag="c")
                        nc.vector.tensor_mul(c_t[:], si[:], tg[:])
                        if d == 0:
                            # c_last lives at rows b*256+255 -> batches 2n,2n+1
                            # at in-chunk columns 255 and 511
                            for j in range(2):
                                b_ = 2 * n + j
                                col = j * 256 + 255
                                nc.vector.tensor_copy(
                                    out=c0lt[:, hm * NB + b_: hm * NB + b_ + 1],
                                    in_=c_t[:, col:col + 1])
                        tc_t = act.tile([P, 512], F32, tag="tc")
                        nc.scalar.activation(tc_t[:], c_t[:], TANH)
                        so = act.tile([P, 512], F32, tag="so")
                        nc.scalar.activation(so[:], ps3[2][:], SIG,
                                             bias=b0sum[d][:, 8 + hm:9 + hm])
                        kk = d * 4 + hm
                        h_ap = inp1T[:, kk * R + n * 512: kk * R + (n + 1) * 512]
                        nc.vector.tensor_mul(h_ap, so[:], tc_t[:])
                        if d == 1:
                            # enc0 chunk ready as soon as both dirs done: gives
                            # the PE transpose work during the L0->L1 handoff
                            base = hm * R + n * 512
                            nc.vector.tensor_add(
                                enc0T[:, base:base + 512],
                                inp1T[:, base:base + 512].bitcast(F32),
                                inp1T[:, KE * R + base: KE * R + base + 512].bitcast(F32))

        # phase 4: enc0 -> row-major -> DMA
        for m in range(NM):
            rm = outp.tile([P, H], F32, tag="rm")
            tp = pst.tile([P, 512], F32, tag="tp")
            for k in range(KE):
                nc.tensor.transpose(out=tp[:, k * P:(k + 1) * P],
                                    in_=enc0T[:, k * R + m * P: k * R + (m + 1) * P],
                                    identity=ident[:])
            nc.vector.tensor_copy(out=rm[:], in_=tp[:])
            nc.sync.dma_start(out=enc0_d[m * P:(m + 1) * P, :], in_=rm[:])

        # phase 5: layer-0 h_last
        for k in range(KE):
            for b_ in range(NB):
                col = k * R + b_ * T + (T - 1)
                nc.vector.tensor_copy(out=h0lt[:, k * NB + b_: k * NB + b_ + 1],
                                      in_=inp1T[:, col:col + 1].bitcast(F32))
        nc.sync.dma_start(out=h0l_d[:], in_=h0lt[:])
        nc.sync.dma_start(out=c0l_d[:], in_=c0lt[:])

        # phase 6: layer 1
        with ExitStack() as scope_c:
            w1pool = scope_c.enter_context(tc.tile_pool(name="w1pool", bufs=1))
            for d in range(2):
                w1 = []
                for k in range(K1):
                    if d == 0 and k < 2:
                        w1.append(w1pre_t[k])
                        continue
                    wt = w1pool.tile([P, G], F32R, tag=f"w1_{k}", name=f"w1_{d}_{k}")
                    nc.sync.dma_start(out=wt[:], in_=wt1_d[d, k * P:(k + 1) * P, :])
                    w1.append(wt)
                for m in range(NM):
                    ps3 = [None] * 3
                    for k in range(K1):
                        for gate in range(3):
                            if k == 0:
                                ps3[gate] = psg.tile([P, 512], F32, tag="ps", name=f"ps1_{gate}")
                            nc.tensor.matmul(
                                out=ps3[gate][:],
                                lhsT=inp1T[:, k * R + m * P: k * R + (m + 1) * P],
                                rhs=w1[k][:, gate * 512:(gate + 1) * 512],
                                start=(k == 0), stop=(k == K1 - 1),
                            )
                    gs = []  # i, g, o post-bias gate tiles [128 rows, 512]
                    for gate, tag in ((0, "si"), (1, "tg"), (2, "so")):
                        g_t = act.tile([P, 512], F32, tag=tag)
                        nc.vector.tensor_add(g_t[:], ps3[gate][:],
                                             b128[d][:, gate * 512:(gate + 1) * 512])
                        gs.append(g_t)
                    # sigmoid/tanh applied in place on the gate tiles
                    nc.scalar.activation(gs[0][:], gs[0][:], SIG)
                    nc.scalar.activation(gs[1][:], gs[1][:], TANH)
                    c_t = act.tile([P, 512], F32, tag="c")
                    nc.vector.tensor_mul(c_t[:], gs[0][:], gs[1][:])
                    tc_t = act.tile([P, 512], F32, tag="tc")
                    nc.scalar.activation(tc_t[:], c_t[:], TANH)
                    nc.scalar.activation(gs[2][:], gs[2][:], SIG)
                    if d == 0:
                        h_ap = h1f_all[:, m * H:(m + 1) * H]
                        nc.vector.tensor_mul(h_ap, gs[2][:], tc_t[:])
                        if m % 2 == 1:  # row m*128+127 = batch (m-1)//2 last step
                            b_ = (m - 1) // 2
                            nc.sync.dma_start(out=h1l_d[b_:b_ + 1, :],
                                              in_=h1f_all[127:128, m * H:(m + 1) * H])
                            nc.sync.dma_start(out=c1l_d[b_:b_ + 1, :],
                                              in_=c_t[127:128, :])
                    else:
                        h1b = act.tile([P, 512], F32, tag="c")
                        nc.vector.tensor_mul(h1b[:], gs[2][:], tc_t[:])
                        e1 = outp.tile([P, H], F32, tag="e1")
                        nc.vector.tensor_add(e1[:], h1f_all[:, m * H:(m + 1) * H],
                                             h1b[:])
                        nc.sync.dma_start(out=enc1_d[m * P:(m + 1) * P, :], in_=e1[:])

    nc.compile()
    return nc


def _prep_inputs(x, emb, W_ih_l0, b_ih_l0, b_hh_l0, W_ih_l1, b_ih_l1, b_hh_l1):
    """Host-side layout prep (slicing / transposition only) + per-core shards."""
    igo = np.r_[0:H, 2 * H:3 * H, 3 * H:4 * H]
    wt0 = np.ascontiguousarray(W_ih_l0[:, igo, :].transpose(0, 2, 1))  # (2, E, G)
    wt1 = np.ascontiguousarray(W_ih_l1[:, igo, :].transpose(0, 2, 1))  # (2, 2H, G)
    b0i = np.ascontiguousarray(b_ih_l0[:, igo])
    b0h = np.ascontiguousarray(b_hh_l0[:, igo])
    b1i = np.ascontiguousarray(b_ih_l1[:, igo])
    b1h = np.ascontiguousarray(b_hh_l1[:, igo])
    in_maps = []
    for i in range(NCORES):
        x_i = np.ascontiguousarray(
            x[i * NB:(i + 1) * NB].reshape(NM, P).astype(np.int32))
        in_maps.append({
            "x": x_i, "emb": emb,
            "wt0": wt0, "b0i": b0i, "b0h": b0h,
            "wt1": wt1, "b1i": b1i, "b1h": b1h,
        })
    return in_maps


def kernel(x, emb, W_ih_l0, W_hh_l0, b_ih_l0, b_hh_l0,
           W_ih_l1, W_hh_l1, b_ih_l1, b_hh_l1):
    global _PROGRAM, LAST_RESULTS
    if _PROGRAM is None:
        _PROGRAM = _build_program()
    nc = _PROGRAM
    in_maps = _prep_inputs(np.asarray(x),
                           np.ascontiguousarray(emb, dtype=np.float32),
                           np.asarray(W_ih_l0, dtype=np.float32),
                           np.asarray(b_ih_l0, dtype=np.float32),
                           np.asarray(b_hh_l0, dtype=np.float32),
                           np.asarray(W_ih_l1, dtype=np.float32),
                           np.asarray(b_ih_l1, dtype=np.float32),
                           np.asarray(b_hh_l1, dtype=np.float32))
    trace = bool(int(os.environ.get("KERNEL_TRACE", "0")))
    res = run_bass_kernel_spmd(nc, in_maps, core_ids=list(range(NCORES)),
                               trace=trace)
    LAST_RESULTS = res

    enc = np.empty((B, T, 2, H), dtype=np.float32)
    h_last = np.empty((2, B, H), dtype=np.float32)
    c_last = np.empty((2, B, H), dtype=np.float32)
    for i in range(NCORES):
        r = res.results[i]
        bs = slice(i * NB, (i + 1) * NB)
        enc[bs, :, 0, :] = r["enc0"].reshape(NB, T, H)
        enc[bs, :, 1, :] = r["enc1"].reshape(NB, T, H)
        # h0l/c0l: [p, k*NB+b] -> [b, k*128+p]
        h_last[0, bs] = r["h0l"].reshape(P, KE, NB).transpose(2, 1, 0).reshape(NB, H)
        c_last[0, bs] = r["c0l"].reshape(P, KE, NB).transpose(2, 1, 0).reshape(NB, H)
        h_last[1, bs] = r["h1l"]
        c_last[1, bs] = r["c1l"]
    return h_last, c_last, enc
